# revision 21
# baseline (speedup 1.0000x reference)
"""Multi-head self-attention (B=2, S=2048, D=1024, H=16, causal) on 8 trn2 cores.

Sharding: batch x heads. Core c owns batch c//4 and heads
[4*(c%4), 4*(c%4)+4) as two head-pairs. Each core returns a partial
[2048, 1024] output (its heads' attention projected through its slice
of o_proj); the host sums 4 partials per batch.

Per-core kernel (all operands bf16, f32 PSUM accumulation):
  - x arrives pre-transposed and bf16-cast (xt [1024, 2048]), loaded
    once (column-quartered DMAs so the first projection chunk starts
    after ~1MB) and reused by both head-pairs' projections.
  - projections per pair: qt/kt [128, 2048] bf16 (2 heads stacked on
    partitions); vt transposed via PE into vg [tokens, j, 66] with a
    ones column so the AV matmul also produces the softmax denominator.
  - scores in transposed layout scT[k, q] = K @ Q^T; both heads of a
    key-tile j write one [128, 1024] 2-bank PSUM tile (cols 0:512 h0,
    512:1024 h1) so ONE ACT exp covers both heads — halving the
    352-cycle-per-instruction ACT overhead. The two 64-contraction
    score matmuls co-execute in PE row groups (0,0)/(64,0). Causal
    staircase skips invalid columns; triangular mask multiplies only
    diagonal blocks. AV (transposed: avT[65, q], row 64 = denominator)
    trails the scores by 2 iterations.
  - normalize per (head, qc) with near-zero PE cost: DVE reciprocal of
    the PE ones-broadcast denominator, DVE multiply into avt. Emitted
    at the top of the NEXT qc so the broadcast matmul never stalls the
    in-order PE queue.
  - O projection per token-tile accumulates both head-pairs in one
    PSUM group; emitted one qc behind pair 1's attention.
"""

import os
import numpy as np
from contextlib import ExitStack

import ml_dtypes

import concourse.bass as bass
import concourse.tile as tile
from concourse import bacc, mybir
from concourse.bass_utils import run_bass_kernel_spmd

F32 = mybir.dt.float32
F32R = mybir.dt.float32r
BF16 = mybir.dt.bfloat16
EXP = mybir.ActivationFunctionType.Exp

B, S, D = 2, 2048, 1024
NCORES = 8
SCALE = 0.125          # 1/sqrt(64)
NQT = S // 128         # 16 query tiles per core
BF = ml_dtypes.bfloat16

_BUILT = None
LAST_RESULTS = None


def _build():
    nc = bacc.Bacc("TRN2", target_bir_lowering=False, debug=False,
                   num_devices=NCORES)
    xt_d = nc.dram_tensor("xt", [D, S], BF16, kind="ExternalInput").ap()
    wq_d = nc.dram_tensor("wq", [2, 128, D], BF16, kind="ExternalInput").ap()
    wk_d = nc.dram_tensor("wk", [2, 128, D], BF16, kind="ExternalInput").ap()
    wv_d = nc.dram_tensor("wv", [2, 128, D], BF16, kind="ExternalInput").ap()
    wo_d = nc.dram_tensor("wo", [2, 128, D], BF16, kind="ExternalInput").ap()
    tri_d = nc.dram_tensor("tri", [128, 128], BF16, kind="ExternalInput").ap()
    id_d = nc.dram_tensor("ident", [128, 128], BF16, kind="ExternalInput").ap()
    ones1_d = nc.dram_tensor("ones1", [1, 64], F32R, kind="ExternalInput").ap()
    out_d = nc.dram_tensor("out", [S, D], BF16, kind="ExternalOutput").ap()

    with tile.TileContext(nc) as tc, ExitStack() as ctx:
        consts = ctx.enter_context(tc.tile_pool(name="consts", bufs=1))
        sb = ctx.enter_context(tc.tile_pool(name="sb", bufs=1))
        ps = ctx.enter_context(tc.tile_pool(name="ps", bufs=1, space="PSUM"))

        wq_t = [consts.tile([128, D], BF16, tag="wq", bufs=2, name=f"wq{p}")
                for p in range(2)]
        wk_t = [consts.tile([128, D], BF16, tag="wk", bufs=2, name=f"wk{p}")
                for p in range(2)]
        wv_t = [consts.tile([128, D], BF16, tag="wv", bufs=2, name=f"wv{p}")
                for p in range(2)]
        wo_t = [consts.tile([128, D], BF16, tag="wo", bufs=2, name=f"wo{p}")
                for p in range(2)]
        tri_t = consts.tile([128, 128], BF16, tag="tri")
        id_t = consts.tile([128, 128], BF16, tag="ident")
        ones1_t = consts.tile([1, 64], F32R, tag="ones1")

        # first projection matmul only needs wq0 + the first x quarter
        nc.sync.dma_start(wq_t[0], wq_d[0])
        xth = [sb.tile([128, S], BF16, tag="xt", bufs=8, name=f"xt{k}")
               for k in range(8)]
        for half in range(2):
            cq = slice(1024 * half, 1024 * (half + 1))
            for k in range(8):
                nc.sync.dma_start(xth[k][:, cq],
                                  xt_d[128 * k:128 * (k + 1), cq])
        nc.sync.dma_start(wk_t[0], wk_d[0])
        nc.sync.dma_start(wv_t[0], wv_d[0])
        nc.gpsimd.dma_start(tri_t, tri_d)
        nc.gpsimd.dma_start(id_t, id_d)
        nc.gpsimd.dma_start(ones1_t, ones1_d)
        nc.sync.dma_start(wq_t[1], wq_d[1])
        nc.sync.dma_start(wk_t[1], wk_d[1])
        nc.sync.dma_start(wv_t[1], wv_d[1])
        nc.sync.dma_start(wo_t[0], wo_d[0])
        nc.sync.dma_start(wo_t[1], wo_d[1])

        qt = [None, None]
        kt = [None, None]
        vg = [[None, None], [None, None]]
        avt = [None, None]

        def project(p):
            def one(w_t, tag):
                dst = sb.tile([128, S], BF16, tag=tag, bufs=2,
                              name=f"{tag}{p}")
                for chk in range(4):
                    pp = ps.tile([128, 1024], F32, tag="sc", bufs=2)
                    for k in range(8):
                        nc.tensor.matmul(
                            pp[:, 0:512],
                            lhsT=w_t[:, 128 * k:128 * (k + 1)],
                            rhs=xth[k][:, 512 * chk:512 * (chk + 1)],
                            start=(k == 0), stop=(k == 7))
                    nc.scalar.copy(dst[:, 512 * chk:512 * (chk + 1)],
                                   pp[:, 0:512])
                return dst

            qt[p] = one(wq_t[p], "qt")
            kt[p] = one(wk_t[p], "kt")
            vt = one(wv_t[p], "vt")
            for h in range(2):
                vgh = sb.tile([128, NQT, 66], BF16, tag=f"vg{h}", bufs=2,
                              name=f"vg{p}_{h}")
                nc.gpsimd.memset(vgh[:, :, 64:65], 1.0)
                vg[p][h] = vgh
            for j in range(NQT):
                tp = ps.tile([128, 128], BF16, tag="tp", bufs=1)
                nc.tensor.transpose(tp, vt[:, 128 * j:128 * (j + 1)], id_t)
                nc.vector.tensor_copy(vg[p][0][:, j, 0:64], tp[:, 0:64])
                nc.vector.tensor_copy(vg[p][1][:, j, 0:64], tp[:, 64:128])

        def o_unit(tt):
            ost = sb.tile([128, D], BF16, tag="ost", bufs=2, name=f"ost{tt}")
            for chv in range(2):
                op = ps.tile([128, 1024], F32, tag="sc", bufs=2)
                nc.tensor.matmul(
                    op[:, 0:512], lhsT=avt[0][:, 128 * tt:128 * (tt + 1)],
                    rhs=wo_t[0][:, 512 * chv:512 * (chv + 1)],
                    start=True, stop=False, skip_group_check=True)
                nc.tensor.matmul(
                    op[:, 0:512], lhsT=avt[1][:, 128 * tt:128 * (tt + 1)],
                    rhs=wo_t[1][:, 512 * chv:512 * (chv + 1)],
                    start=False, stop=True, skip_group_check=True)
                nc.scalar.copy(ost[:, 512 * chv:512 * (chv + 1)],
                               op[:, 0:512])
            nc.sync.dma_start(out_d[128 * tt:128 * (tt + 1), :], ost)

        avps_of = {}
        dens_of = {}

        def norm_den(p, qc):
            """Denominator rows -> SBUF (DVE), emitted at the top of
            the next qc so the DVE queue clears well before norm_bc."""
            avps = avps_of[(p, qc)]
            dens = []
            with tc.high_priority():
                for h in range(2):
                    den = sb.tile([1, 512], F32R, tag="den", bufs=4)
                    nc.scalar.copy(den, avps[h][64:65, :])
                    dens.append(den)
            dens_of[(p, qc)] = dens

        def norm_bc(p, qc):
            """avt[p][64h:64h+64, qc cols] = avps[h][0:64] / row64.
            Emitted mid-next-qc as a filler: den long ready, so the
            PE-queue broadcast matmul never stalls."""
            avps = avps_of[(p, qc)]
            dens = dens_of[(p, qc)]
            cs = slice(512 * qc, 512 * (qc + 1))
            for h in range(2):
                rb_ps = ps.tile([64, 512], F32, tag="tp", bufs=1)
                nc.tensor.matmul(rb_ps, lhsT=ones1_t, rhs=dens[h],
                                 start=True, stop=True)
                rbs = sb.tile([64, 512], F32, tag="rb", bufs=2)
                nc.vector.reciprocal(rbs, rb_ps)
                nc.vector.tensor_mul(avt[p][64 * h:64 * (h + 1), cs],
                                     avps[h][0:64, :], rbs)

        def attention(p, pre, fillers):
            """pre[qc]: thunks emitted before qc's j-loop (normalizes).
            fillers[qc]: thunks spread through qc's j-loop (O units)."""
            avt[p] = sb.tile([128, S], BF16, tag="avt", bufs=2,
                             name=f"avt{p}")
            for qc in range(4):
                njt = 4 * qc + 4
                avps = [ps.tile([65, 512], F32, tag="av", bufs=3,
                                name=f"av{p}_{qc}_{h}")
                        for h in range(2)]
                avps_of[(p, qc)] = avps
                fl = fillers[qc]
                nfl = len(fl)
                pend = []

                def do_av(j, et2):
                    vs = max(0, 128 * (j - 4 * qc))
                    for h in range(2):
                        nc.tensor.matmul(
                            avps[h][:, vs:512],
                            lhsT=vg[p][h][:, j, 0:65],
                            rhs=et2[:, 512 * h + vs:512 * (h + 1)],
                            start=(j == 0), stop=(j == njt - 1),
                            skip_group_check=True)

                for j in range(njt):
                    vs = max(0, 128 * (j - 4 * qc))
                    sc2 = ps.tile([128, 1024], F32, tag="sc", bufs=2)
                    for h in range(2):
                        nc.tensor.matmul(
                            sc2[:, 512 * h + vs:512 * (h + 1)],
                            lhsT=kt[p][64 * h:64 * (h + 1),
                                       128 * j:128 * (j + 1)],
                            rhs=qt[p][64 * h:64 * (h + 1),
                                      512 * qc + vs:512 * (qc + 1)],
                            start=True, stop=True)
                    et2 = sb.tile([128, 1024], BF16, tag="et", bufs=8)
                    if vs < 384:
                        # one ACT instruction covers both heads (the
                        # gap cols [512:512+vs] hold stale-but-finite
                        # scores; their exp lands in unread et2 cols)
                        nc.scalar.activation(et2[:, vs:1024],
                                             sc2[:, vs:1024],
                                             EXP, scale=SCALE)
                    else:
                        for h in range(2):
                            nc.scalar.activation(
                                et2[:, 512 * h + vs:512 * (h + 1)],
                                sc2[:, 512 * h + vs:512 * (h + 1)],
                                EXP, scale=SCALE)
                    if j >= 4 * qc:
                        for h in range(2):
                            nc.vector.tensor_mul(
                                et2[:, 512 * h + vs:512 * h + vs + 128],
                                et2[:, 512 * h + vs:512 * h + vs + 128],
                                tri_t)
                    pend.append((j, et2))
                    if j == 2:
                        # deferred normalizes ride behind two queued
                        # score-pairs so their broadcast matmul meets a
                        # long-retired denominator copy
                        for th in pre[qc]:
                            th()
                    if len(pend) > 2:   # AV trails scores by 2 iterations
                        do_av(*pend.pop(0))
                    # fillers start at j==3: they may read avt columns
                    # that the j==2 deferred normalize produces
                    if j >= 3:
                        fj = j - 3
                        k0 = nfl * fj // (njt - 3)
                        k1 = nfl * (fj + 1) // (njt - 3)
                        for k in range(k0, k1):
                            fl[k]()
                for args in pend:
                    do_av(*args)
                norm_den(p, qc)

        project(0)
        project(1)
        attention(0,
                  pre=[[],
                       [lambda: norm_bc(0, 0)],
                       [lambda: norm_bc(0, 1)],
                       [lambda: norm_bc(0, 2)]],
                  fillers=[[], [], [], []])
        # pair-1 attention carries pair-0's last normalize, its own
        # normalizes, and the O projection (one qc behind)
        attention(1,
                  pre=[[lambda: norm_bc(0, 3)],
                       [lambda: norm_bc(1, 0)],
                       [lambda: norm_bc(1, 1)],
                       [lambda: norm_bc(1, 2)]],
                  fillers=[
                      [],
                      [lambda tt=t: o_unit(tt) for t in range(0, 4)],
                      [lambda tt=t: o_unit(tt) for t in range(4, 8)],
                      [lambda tt=t: o_unit(tt) for t in range(8, 12)],
                  ])
        norm_bc(1, 3)
        for tt in range(12, 16):
            o_unit(tt)
    nc.compile()
    return nc


def _get_built():
    global _BUILT
    if _BUILT is None:
        _BUILT = _build()
    return _BUILT


def _host_inputs(x, q_proj, k_proj, v_proj, o_proj):
    tri = np.triu(np.ones((128, 128), dtype=np.float32)).astype(BF)
    ident = np.eye(128, dtype=np.float32).astype(BF)
    xt = [np.ascontiguousarray(x[b].T).astype(BF) for b in range(B)]

    def wslice(w, gp):
        # [p, 8k x 128m]: w_sb[p, 128k+m] = w[128gp+m, 128k+p]
        a = w[128 * gp:128 * (gp + 1)].reshape(128, 8, 128)
        return np.ascontiguousarray(a.transpose(2, 1, 0).reshape(128, D))

    in_maps = []
    for c in range(NCORES):
        b, g4 = c // 4, c % 4
        gps = (2 * g4, 2 * g4 + 1)
        wq = np.stack([wslice(q_proj, gp) for gp in gps]).astype(BF)
        wk = np.stack([wslice(k_proj, gp) for gp in gps]).astype(BF)
        wv = np.stack([wslice(v_proj, gp) for gp in gps]).astype(BF)
        wo = np.stack(
            [np.ascontiguousarray(o_proj[:, 128 * gp:128 * (gp + 1)].T)
             for gp in gps]).astype(BF)
        in_maps.append(dict(xt=xt[b], wq=wq, wk=wk, wv=wv, wo=wo,
                            tri=tri, ident=ident,
                            ones1=np.ones((1, 64), dtype=np.float32)))
    return in_maps


def kernel(**inputs):
    x = np.asarray(inputs["x"], dtype=np.float32)
    q_proj = np.asarray(inputs["q_proj"], dtype=np.float32)
    k_proj = np.asarray(inputs["k_proj"], dtype=np.float32)
    v_proj = np.asarray(inputs["v_proj"], dtype=np.float32)
    o_proj = np.asarray(inputs["o_proj"], dtype=np.float32)

    in_maps = _host_inputs(x, q_proj, k_proj, v_proj, o_proj)
    nc = _get_built()
    global LAST_RESULTS
    LAST_RESULTS = run_bass_kernel_spmd(
        nc, in_maps, core_ids=list(range(NCORES)),
        trace=bool(os.environ.get("KERNEL_TRACE")))
    y = np.zeros((B, S, D), dtype=np.float32)
    for c in range(NCORES):
        y[c // 4] += np.asarray(LAST_RESULTS.results[c]["out"]).astype(
            np.float32)
    return y


# revision 22
# speedup vs baseline: 1.0005x; 1.0005x over previous
"""Multi-head self-attention (B=2, S=2048, D=1024, H=16, causal) on 8 trn2 cores.

Sharding: batch x heads. Core c owns batch c//4 and heads
[4*(c%4), 4*(c%4)+4) as two head-pairs. Each core returns a partial
[2048, 1024] output (its heads' attention projected through its slice
of o_proj); the host sums 4 partials per batch.

Per-core kernel (all operands bf16, f32 PSUM accumulation):
  - x arrives pre-transposed and bf16-cast (xt [1024, 2048]), loaded
    once (column-quartered DMAs so the first projection chunk starts
    after ~1MB) and reused by both head-pairs' projections.
  - projections per pair: qt/kt [128, 2048] bf16 (2 heads stacked on
    partitions); vt transposed via PE into vg [tokens, j, 66] with a
    ones column so the AV matmul also produces the softmax denominator.
  - scores in transposed layout scT[k, q] = K @ Q^T; both heads of a
    key-tile j write one [128, 1024] 2-bank PSUM tile (cols 0:512 h0,
    512:1024 h1) so ONE ACT exp covers both heads — halving the
    352-cycle-per-instruction ACT overhead. The two 64-contraction
    score matmuls co-execute in PE row groups (0,0)/(64,0). Causal
    staircase skips invalid columns; triangular mask multiplies only
    diagonal blocks. AV (transposed: avT[65, q], row 64 = denominator)
    trails the scores by 2 iterations.
  - normalize per (head, qc) with near-zero PE cost: DVE reciprocal of
    the PE ones-broadcast denominator, DVE multiply into avt. Emitted
    at the top of the NEXT qc so the broadcast matmul never stalls the
    in-order PE queue.
  - O projection per token-tile accumulates both head-pairs in one
    PSUM group; emitted one qc behind pair 1's attention.
"""

import os
import numpy as np
from contextlib import ExitStack

import ml_dtypes

import concourse.bass as bass
import concourse.tile as tile
from concourse import bacc, mybir
from concourse.bass_utils import run_bass_kernel_spmd

F32 = mybir.dt.float32
F32R = mybir.dt.float32r
BF16 = mybir.dt.bfloat16
EXP = mybir.ActivationFunctionType.Exp

B, S, D = 2, 2048, 1024
NCORES = 8
SCALE = 0.125          # 1/sqrt(64)
NQT = S // 128         # 16 query tiles per core
BF = ml_dtypes.bfloat16

_BUILT = None
LAST_RESULTS = None


def _build():
    nc = bacc.Bacc("TRN2", target_bir_lowering=False, debug=False,
                   num_devices=NCORES)
    xt_d = nc.dram_tensor("xt", [D, S], BF16, kind="ExternalInput").ap()
    wq_d = nc.dram_tensor("wq", [2, 128, D], BF16, kind="ExternalInput").ap()
    wk_d = nc.dram_tensor("wk", [2, 128, D], BF16, kind="ExternalInput").ap()
    wv_d = nc.dram_tensor("wv", [2, 128, D], BF16, kind="ExternalInput").ap()
    wo_d = nc.dram_tensor("wo", [2, 128, D], BF16, kind="ExternalInput").ap()
    tri_d = nc.dram_tensor("tri", [128, 128], BF16, kind="ExternalInput").ap()
    id_d = nc.dram_tensor("ident", [128, 128], BF16, kind="ExternalInput").ap()
    ones1_d = nc.dram_tensor("ones1", [1, 64], F32R, kind="ExternalInput").ap()
    out_d = nc.dram_tensor("out", [S, D], BF16, kind="ExternalOutput").ap()

    with tile.TileContext(nc) as tc, ExitStack() as ctx:
        consts = ctx.enter_context(tc.tile_pool(name="consts", bufs=1))
        sb = ctx.enter_context(tc.tile_pool(name="sb", bufs=1))
        ps = ctx.enter_context(tc.tile_pool(name="ps", bufs=1, space="PSUM"))

        wq_t = [consts.tile([128, D], BF16, tag="wq", bufs=2, name=f"wq{p}")
                for p in range(2)]
        wk_t = [consts.tile([128, D], BF16, tag="wk", bufs=2, name=f"wk{p}")
                for p in range(2)]
        wv_t = [consts.tile([128, D], BF16, tag="wv", bufs=2, name=f"wv{p}")
                for p in range(2)]
        wo_t = [consts.tile([128, D], BF16, tag="wo", bufs=2, name=f"wo{p}")
                for p in range(2)]
        tri_t = consts.tile([128, 128], BF16, tag="tri")
        id_t = consts.tile([128, 128], BF16, tag="ident")
        ones1_t = consts.tile([1, 64], F32R, tag="ones1")

        # first projection matmul only needs wq0 + the first x quarter
        nc.sync.dma_start(wq_t[0], wq_d[0])
        xth = [sb.tile([128, S], BF16, tag="xt", bufs=8, name=f"xt{k}")
               for k in range(8)]
        for half in range(2):
            cq = slice(1024 * half, 1024 * (half + 1))
            for k in range(8):
                nc.sync.dma_start(xth[k][:, cq],
                                  xt_d[128 * k:128 * (k + 1), cq])
        nc.sync.dma_start(wk_t[0], wk_d[0])
        nc.sync.dma_start(wv_t[0], wv_d[0])
        nc.gpsimd.dma_start(tri_t, tri_d)
        nc.gpsimd.dma_start(id_t, id_d)
        nc.gpsimd.dma_start(ones1_t, ones1_d)
        nc.sync.dma_start(wq_t[1], wq_d[1])
        nc.sync.dma_start(wk_t[1], wk_d[1])
        nc.sync.dma_start(wv_t[1], wv_d[1])
        nc.sync.dma_start(wo_t[0], wo_d[0])
        nc.sync.dma_start(wo_t[1], wo_d[1])

        qt = [None, None]
        kt = [None, None]
        vg = [[None, None], [None, None]]
        avt = [None, None]

        def project(p):
            def one(w_t, tag):
                dst = sb.tile([128, S], BF16, tag=tag, bufs=2,
                              name=f"{tag}{p}")
                for chk in range(4):
                    pp = ps.tile([128, 1024], F32, tag="sc", bufs=2)
                    for k in range(8):
                        nc.tensor.matmul(
                            pp[:, 0:512],
                            lhsT=w_t[:, 128 * k:128 * (k + 1)],
                            rhs=xth[k][:, 512 * chk:512 * (chk + 1)],
                            start=(k == 0), stop=(k == 7))
                    nc.scalar.copy(dst[:, 512 * chk:512 * (chk + 1)],
                                   pp[:, 0:512])
                return dst

            qt[p] = one(wq_t[p], "qt")
            kt[p] = one(wk_t[p], "kt")
            vt = one(wv_t[p], "vt")
            for h in range(2):
                vgh = sb.tile([128, NQT, 66], BF16, tag=f"vg{h}", bufs=2,
                              name=f"vg{p}_{h}")
                nc.gpsimd.memset(vgh[:, :, 64:65], 1.0)
                vg[p][h] = vgh
            for j in range(NQT):
                tp = ps.tile([128, 128], BF16, tag="tp", bufs=1)
                nc.tensor.transpose(tp, vt[:, 128 * j:128 * (j + 1)], id_t)
                nc.vector.tensor_copy(vg[p][0][:, j, 0:64], tp[:, 0:64])
                nc.vector.tensor_copy(vg[p][1][:, j, 0:64], tp[:, 64:128])

        def o_unit(tt):
            ost = sb.tile([128, D], BF16, tag="ost", bufs=2, name=f"ost{tt}")
            for chv in range(2):
                op = ps.tile([128, 1024], F32, tag="sc", bufs=2)
                nc.tensor.matmul(
                    op[:, 0:512], lhsT=avt[0][:, 128 * tt:128 * (tt + 1)],
                    rhs=wo_t[0][:, 512 * chv:512 * (chv + 1)],
                    start=True, stop=False, skip_group_check=True)
                nc.tensor.matmul(
                    op[:, 0:512], lhsT=avt[1][:, 128 * tt:128 * (tt + 1)],
                    rhs=wo_t[1][:, 512 * chv:512 * (chv + 1)],
                    start=False, stop=True, skip_group_check=True)
                nc.scalar.copy(ost[:, 512 * chv:512 * (chv + 1)],
                               op[:, 0:512])
            nc.sync.dma_start(out_d[128 * tt:128 * (tt + 1), :], ost)

        avps_of = {}
        dens_of = {}

        def norm_den(p, qc):
            """Denominator rows -> SBUF (DVE), emitted at the top of
            the next qc so the DVE queue clears well before norm_bc."""
            avps = avps_of[(p, qc)]
            dens = []
            with tc.high_priority():
                for h in range(2):
                    den = sb.tile([1, 512], F32R, tag="den", bufs=4)
                    nc.scalar.copy(den, avps[h][64:65, :])
                    dens.append(den)
            dens_of[(p, qc)] = dens

        def norm_bc(p, qc):
            """avt[p][64h:64h+64, qc cols] = avps[h][0:64] / row64.
            Emitted mid-next-qc as a filler: den long ready, so the
            PE-queue broadcast matmul never stalls."""
            avps = avps_of[(p, qc)]
            dens = dens_of[(p, qc)]
            cs = slice(512 * qc, 512 * (qc + 1))
            for h in range(2):
                rb_ps = ps.tile([64, 512], F32, tag="tp", bufs=1)
                nc.tensor.matmul(rb_ps, lhsT=ones1_t, rhs=dens[h],
                                 start=True, stop=True)
                rbs = sb.tile([64, 512], F32, tag="rb", bufs=2)
                nc.vector.reciprocal(rbs, rb_ps)
                nc.vector.tensor_mul(avt[p][64 * h:64 * (h + 1), cs],
                                     avps[h][0:64, :], rbs)

        def attention(p, pre, fillers):
            """pre[qc]: thunks emitted before qc's j-loop (normalizes).
            fillers[qc]: thunks spread through qc's j-loop (O units)."""
            avt[p] = sb.tile([128, S], BF16, tag="avt", bufs=2,
                             name=f"avt{p}")
            for qc in range(4):
                njt = 4 * qc + 4
                avps = [ps.tile([65, 512], F32, tag="av", bufs=3,
                                name=f"av{p}_{qc}_{h}")
                        for h in range(2)]
                avps_of[(p, qc)] = avps
                fl = fillers[qc]
                nfl = len(fl)
                pend = []

                def do_av(j, et2):
                    vs = max(0, 128 * (j - 4 * qc))
                    for h in range(2):
                        nc.tensor.matmul(
                            avps[h][:, vs:512],
                            lhsT=vg[p][h][:, j, 0:65],
                            rhs=et2[:, 512 * h + vs:512 * (h + 1)],
                            start=(j == 0), stop=(j == njt - 1),
                            skip_group_check=True)

                for j in range(njt):
                    vs = max(0, 128 * (j - 4 * qc))
                    sc2 = ps.tile([128, 1024], F32, tag="sc", bufs=2)
                    for h in range(2):
                        nc.tensor.matmul(
                            sc2[:, 512 * h + vs:512 * (h + 1)],
                            lhsT=kt[p][64 * h:64 * (h + 1),
                                       128 * j:128 * (j + 1)],
                            rhs=qt[p][64 * h:64 * (h + 1),
                                      512 * qc + vs:512 * (qc + 1)],
                            start=True, stop=True)
                    et2 = sb.tile([128, 1024], BF16, tag="et", bufs=8)
                    if vs < 384:
                        # one ACT instruction covers both heads (the
                        # gap cols [512:512+vs] hold stale-but-finite
                        # scores; their exp lands in unread et2 cols)
                        nc.scalar.activation(et2[:, vs:1024],
                                             sc2[:, vs:1024],
                                             EXP, scale=SCALE)
                    else:
                        for h in range(2):
                            nc.scalar.activation(
                                et2[:, 512 * h + vs:512 * (h + 1)],
                                sc2[:, 512 * h + vs:512 * (h + 1)],
                                EXP, scale=SCALE)
                    if j >= 4 * qc:
                        for h in range(2):
                            nc.vector.tensor_mul(
                                et2[:, 512 * h + vs:512 * h + vs + 128],
                                et2[:, 512 * h + vs:512 * h + vs + 128],
                                tri_t)
                    pend.append((j, et2))
                    if j == 3:
                        # deferred normalizes ride behind two queued
                        # score-pairs so their broadcast matmul meets a
                        # long-retired denominator copy
                        for th in pre[qc]:
                            th()
                    if len(pend) > 3:   # AV trails scores by 3 iterations
                        do_av(*pend.pop(0))
                    # fillers start at j==4: they may read avt columns
                    # that the j==3 deferred normalize produces
                    if nfl and j >= 4:
                        fj = j - 4
                        k0 = nfl * fj // (njt - 4)
                        k1 = nfl * (fj + 1) // (njt - 4)
                        for k in range(k0, k1):
                            fl[k]()
                for args in pend:
                    do_av(*args)
                norm_den(p, qc)

        project(0)
        project(1)
        attention(0,
                  pre=[[],
                       [lambda: norm_bc(0, 0)],
                       [lambda: norm_bc(0, 1)],
                       [lambda: norm_bc(0, 2)]],
                  fillers=[[], [], [], []])
        # pair-1 attention carries pair-0's last normalize, its own
        # normalizes, and the O projection (one qc behind)
        attention(1,
                  pre=[[lambda: norm_bc(0, 3)],
                       [lambda: norm_bc(1, 0)],
                       [lambda: norm_bc(1, 1)],
                       [lambda: norm_bc(1, 2)]],
                  fillers=[
                      [],
                      [lambda tt=t: o_unit(tt) for t in range(0, 4)],
                      [lambda tt=t: o_unit(tt) for t in range(4, 8)],
                      [lambda tt=t: o_unit(tt) for t in range(8, 12)],
                  ])
        norm_bc(1, 3)
        for tt in range(12, 16):
            o_unit(tt)
    nc.compile()
    return nc


def _get_built():
    global _BUILT
    if _BUILT is None:
        _BUILT = _build()
    return _BUILT


def _host_inputs(x, q_proj, k_proj, v_proj, o_proj):
    tri = np.triu(np.ones((128, 128), dtype=np.float32)).astype(BF)
    ident = np.eye(128, dtype=np.float32).astype(BF)
    xt = [np.ascontiguousarray(x[b].T).astype(BF) for b in range(B)]

    def wslice(w, gp):
        # [p, 8k x 128m]: w_sb[p, 128k+m] = w[128gp+m, 128k+p]
        a = w[128 * gp:128 * (gp + 1)].reshape(128, 8, 128)
        return np.ascontiguousarray(a.transpose(2, 1, 0).reshape(128, D))

    in_maps = []
    for c in range(NCORES):
        b, g4 = c // 4, c % 4
        gps = (2 * g4, 2 * g4 + 1)
        wq = np.stack([wslice(q_proj, gp) for gp in gps]).astype(BF)
        wk = np.stack([wslice(k_proj, gp) for gp in gps]).astype(BF)
        wv = np.stack([wslice(v_proj, gp) for gp in gps]).astype(BF)
        wo = np.stack(
            [np.ascontiguousarray(o_proj[:, 128 * gp:128 * (gp + 1)].T)
             for gp in gps]).astype(BF)
        in_maps.append(dict(xt=xt[b], wq=wq, wk=wk, wv=wv, wo=wo,
                            tri=tri, ident=ident,
                            ones1=np.ones((1, 64), dtype=np.float32)))
    return in_maps


def kernel(**inputs):
    x = np.asarray(inputs["x"], dtype=np.float32)
    q_proj = np.asarray(inputs["q_proj"], dtype=np.float32)
    k_proj = np.asarray(inputs["k_proj"], dtype=np.float32)
    v_proj = np.asarray(inputs["v_proj"], dtype=np.float32)
    o_proj = np.asarray(inputs["o_proj"], dtype=np.float32)

    in_maps = _host_inputs(x, q_proj, k_proj, v_proj, o_proj)
    nc = _get_built()
    global LAST_RESULTS
    LAST_RESULTS = run_bass_kernel_spmd(
        nc, in_maps, core_ids=list(range(NCORES)),
        trace=bool(os.environ.get("KERNEL_TRACE")))
    y = np.zeros((B, S, D), dtype=np.float32)
    for c in range(NCORES):
        y[c // 4] += np.asarray(LAST_RESULTS.results[c]["out"]).astype(
            np.float32)
    return y


# revision 23
# speedup vs baseline: 1.0405x; 1.0399x over previous
"""Multi-head self-attention (B=2, S=2048, D=1024, H=16, causal) on 8 trn2 cores.

Sharding: batch x heads. Core c owns batch c//4 and heads
[4*(c%4), 4*(c%4)+4) as two head-pairs. Each core returns a partial
[2048, 1024] output (its heads' attention projected through its slice
of o_proj); the host sums 4 partials per batch.

Per-core kernel (all operands bf16, f32 PSUM accumulation):
  - x arrives pre-transposed and bf16-cast (xt [1024, 2048]), loaded
    once (column-quartered DMAs so the first projection chunk starts
    after ~1MB) and reused by both head-pairs' projections.
  - projections per pair: qt/kt [128, 2048] bf16 (2 heads stacked on
    partitions); vt transposed via PE into vg [tokens, j, 66] with a
    ones column so the AV matmul also produces the softmax denominator.
  - scores in transposed layout scT[k, q] = K @ Q^T; both heads of a
    key-tile j write one [128, 1024] 2-bank PSUM tile (cols 0:512 h0,
    512:1024 h1) so ONE ACT exp covers both heads — halving the
    352-cycle-per-instruction ACT overhead. The two 64-contraction
    score matmuls co-execute in PE row groups (0,0)/(64,0). Causal
    staircase skips invalid columns; triangular mask multiplies only
    diagonal blocks. AV (transposed: avT[65, q], row 64 = denominator)
    trails the scores by 2 iterations.
  - normalize per (head, qc) with near-zero PE cost: DVE reciprocal of
    the PE ones-broadcast denominator, DVE multiply into avt. Emitted
    at the top of the NEXT qc so the broadcast matmul never stalls the
    in-order PE queue.
  - O projection per token-tile accumulates both head-pairs in one
    PSUM group; emitted one qc behind pair 1's attention.
"""

import os
import numpy as np
from contextlib import ExitStack

import ml_dtypes

import concourse.bass as bass
import concourse.tile as tile
from concourse import bacc, mybir
from concourse.bass_utils import run_bass_kernel_spmd

F32 = mybir.dt.float32
F32R = mybir.dt.float32r
BF16 = mybir.dt.bfloat16
EXP = mybir.ActivationFunctionType.Exp

B, S, D = 2, 2048, 1024
NCORES = 8
SCALE = 0.125          # 1/sqrt(64)
NQT = S // 128         # 16 query tiles per core
BF = ml_dtypes.bfloat16

_BUILT = None
LAST_RESULTS = None


def _build():
    nc = bacc.Bacc("TRN2", target_bir_lowering=False, debug=False,
                   num_devices=NCORES)
    xt_d = nc.dram_tensor("xt", [D, S], BF16, kind="ExternalInput").ap()
    wq_d = nc.dram_tensor("wq", [2, 128, D], BF16, kind="ExternalInput").ap()
    wk_d = nc.dram_tensor("wk", [2, 128, D], BF16, kind="ExternalInput").ap()
    wv_d = nc.dram_tensor("wv", [2, 128, D], BF16, kind="ExternalInput").ap()
    wo_d = nc.dram_tensor("wo", [2, 128, D], BF16, kind="ExternalInput").ap()
    tri_d = nc.dram_tensor("tri", [128, 128], BF16, kind="ExternalInput").ap()
    id_d = nc.dram_tensor("ident", [128, 128], BF16, kind="ExternalInput").ap()
    ones1_d = nc.dram_tensor("ones1", [1, 64], F32R, kind="ExternalInput").ap()
    out_d = nc.dram_tensor("out", [S, D], BF16, kind="ExternalOutput").ap()

    with tile.TileContext(nc) as tc, ExitStack() as ctx:
        consts = ctx.enter_context(tc.tile_pool(name="consts", bufs=1))
        sb = ctx.enter_context(tc.tile_pool(name="sb", bufs=1))
        ps = ctx.enter_context(tc.tile_pool(name="ps", bufs=1, space="PSUM"))

        wq_t = [consts.tile([128, D], BF16, tag="wq", bufs=2, name=f"wq{p}")
                for p in range(2)]
        wk_t = [consts.tile([128, D], BF16, tag="wk", bufs=2, name=f"wk{p}")
                for p in range(2)]
        wv_t = [consts.tile([128, D], BF16, tag="wv", bufs=2, name=f"wv{p}")
                for p in range(2)]
        wo_t = [consts.tile([128, D], BF16, tag="wo", bufs=2, name=f"wo{p}")
                for p in range(2)]
        tri_t = consts.tile([128, 128], BF16, tag="tri")
        id_t = consts.tile([128, 128], BF16, tag="ident")
        ones1_t = consts.tile([1, 64], F32R, tag="ones1")

        # first projection matmul only needs wq0 + the first x quarter
        nc.sync.dma_start(wq_t[0], wq_d[0])
        xth = [sb.tile([128, S], BF16, tag="xt", bufs=8, name=f"xt{k}")
               for k in range(8)]
        for half in range(2):
            cq = slice(1024 * half, 1024 * (half + 1))
            for k in range(8):
                nc.sync.dma_start(xth[k][:, cq],
                                  xt_d[128 * k:128 * (k + 1), cq])
        nc.sync.dma_start(wk_t[0], wk_d[0])
        nc.sync.dma_start(wv_t[0], wv_d[0])
        nc.gpsimd.dma_start(tri_t, tri_d)
        nc.gpsimd.dma_start(id_t, id_d)
        nc.gpsimd.dma_start(ones1_t, ones1_d)
        nc.sync.dma_start(wq_t[1], wq_d[1])
        nc.sync.dma_start(wk_t[1], wk_d[1])
        nc.sync.dma_start(wv_t[1], wv_d[1])
        nc.sync.dma_start(wo_t[0], wo_d[0])
        nc.sync.dma_start(wo_t[1], wo_d[1])

        qt = [None, None]
        kt = [None, None]
        vg = [[None, None], [None, None]]
        avt = [None, None]

        def project(p):
            def one(w_t, tag):
                dst = sb.tile([128, S], BF16, tag=tag, bufs=2,
                              name=f"{tag}{p}")
                for chk in range(4):
                    pp = ps.tile([128, 1024], F32, tag="sc", bufs=2)
                    for k in range(8):
                        nc.tensor.matmul(
                            pp[:, 0:512],
                            lhsT=w_t[:, 128 * k:128 * (k + 1)],
                            rhs=xth[k][:, 512 * chk:512 * (chk + 1)],
                            start=(k == 0), stop=(k == 7))
                    nc.scalar.copy(dst[:, 512 * chk:512 * (chk + 1)],
                                   pp[:, 0:512])
                return dst

            qt[p] = one(wq_t[p], "qt")
            kt[p] = one(wk_t[p], "kt")
            vt = one(wv_t[p], "vt")
            for h in range(2):
                vgh = sb.tile([128, NQT, 66], BF16, tag=f"vg{h}", bufs=2,
                              name=f"vg{p}_{h}")
                nc.gpsimd.memset(vgh[:, :, 64:65], 1.0)
                vg[p][h] = vgh
            for j in range(NQT):
                tp = ps.tile([128, 128], BF16, tag="tp", bufs=1)
                nc.tensor.transpose(tp, vt[:, 128 * j:128 * (j + 1)], id_t)
                nc.vector.tensor_copy(vg[p][0][:, j, 0:64], tp[:, 0:64])
                nc.vector.tensor_copy(vg[p][1][:, j, 0:64], tp[:, 64:128])

        def o_unit(tt):
            ost = sb.tile([128, D], BF16, tag="ost", bufs=2, name=f"ost{tt}")
            for chv in range(2):
                op = ps.tile([128, 1024], F32, tag="sc", bufs=2)
                nc.tensor.matmul(
                    op[:, 0:512], lhsT=avt[0][:, 128 * tt:128 * (tt + 1)],
                    rhs=wo_t[0][:, 512 * chv:512 * (chv + 1)],
                    start=True, stop=False, skip_group_check=True)
                nc.tensor.matmul(
                    op[:, 0:512], lhsT=avt[1][:, 128 * tt:128 * (tt + 1)],
                    rhs=wo_t[1][:, 512 * chv:512 * (chv + 1)],
                    start=False, stop=True, skip_group_check=True)
                nc.scalar.copy(ost[:, 512 * chv:512 * (chv + 1)],
                               op[:, 0:512])
            nc.sync.dma_start(out_d[128 * tt:128 * (tt + 1), :], ost)

        avps_of = {}
        dens_of = {}

        def norm_den(p, qc):
            """Denominator rows -> SBUF (DVE), emitted at the top of
            the next qc so the DVE queue clears well before norm_bc."""
            avps = avps_of[(p, qc)]
            dens = []
            with tc.high_priority():
                for h in range(2):
                    den = sb.tile([1, 512], F32R, tag="den", bufs=4)
                    nc.vector.tensor_copy(den, avps[h][64:65, :])
                    dens.append(den)
            dens_of[(p, qc)] = dens

        def norm_bc(p, qc):
            """avt[p][64h:64h+64, qc cols] = avps[h][0:64] / row64.
            Emitted mid-next-qc as a filler: den long ready, so the
            PE-queue broadcast matmul never stalls."""
            avps = avps_of[(p, qc)]
            dens = dens_of[(p, qc)]
            cs = slice(512 * qc, 512 * (qc + 1))
            for h in range(2):
                rb_ps = ps.tile([64, 512], F32, tag="tp", bufs=1)
                nc.tensor.matmul(rb_ps, lhsT=ones1_t, rhs=dens[h],
                                 start=True, stop=True)
                rbs = sb.tile([64, 512], F32, tag="rb", bufs=2)
                nc.vector.reciprocal(rbs, rb_ps)
                nc.vector.tensor_mul(avt[p][64 * h:64 * (h + 1), cs],
                                     avps[h][0:64, :], rbs)

        def attention(p, pre, fillers):
            """pre[qc]: thunks emitted before qc's j-loop (normalizes).
            fillers[qc]: thunks spread through qc's j-loop (O units)."""
            avt[p] = sb.tile([128, S], BF16, tag="avt", bufs=2,
                             name=f"avt{p}")
            for qc in range(4):
                njt = 4 * qc + 4
                avps = [ps.tile([65, 512], F32, tag="av", bufs=3,
                                name=f"av{p}_{qc}_{h}")
                        for h in range(2)]
                avps_of[(p, qc)] = avps
                fl = fillers[qc]
                nfl = len(fl)
                pend = []

                def do_av(j, et2):
                    vs = max(0, 128 * (j - 4 * qc))
                    for h in range(2):
                        nc.tensor.matmul(
                            avps[h][:, vs:512],
                            lhsT=vg[p][h][:, j, 0:65],
                            rhs=et2[:, 512 * h + vs:512 * (h + 1)],
                            start=(j == 0), stop=(j == njt - 1),
                            skip_group_check=True)

                for j in range(njt):
                    vs = max(0, 128 * (j - 4 * qc))
                    sc2 = ps.tile([128, 1024], F32, tag="sc", bufs=2)
                    for h in range(2):
                        nc.tensor.matmul(
                            sc2[:, 512 * h + vs:512 * (h + 1)],
                            lhsT=kt[p][64 * h:64 * (h + 1),
                                       128 * j:128 * (j + 1)],
                            rhs=qt[p][64 * h:64 * (h + 1),
                                      512 * qc + vs:512 * (qc + 1)],
                            start=True, stop=True)
                    et2 = sb.tile([128, 1024], BF16, tag="et", bufs=8)
                    if vs < 384:
                        # one ACT instruction covers both heads (the
                        # gap cols [512:512+vs] hold stale-but-finite
                        # scores; their exp lands in unread et2 cols)
                        nc.scalar.activation(et2[:, vs:1024],
                                             sc2[:, vs:1024],
                                             EXP, scale=SCALE)
                    else:
                        for h in range(2):
                            nc.scalar.activation(
                                et2[:, 512 * h + vs:512 * (h + 1)],
                                sc2[:, 512 * h + vs:512 * (h + 1)],
                                EXP, scale=SCALE)
                    if j >= 4 * qc:
                        for h in range(2):
                            nc.vector.tensor_mul(
                                et2[:, 512 * h + vs:512 * h + vs + 128],
                                et2[:, 512 * h + vs:512 * h + vs + 128],
                                tri_t)
                    pend.append((j, et2))
                    if j == 3:
                        # deferred normalizes ride behind two queued
                        # score-pairs so their broadcast matmul meets a
                        # long-retired denominator copy
                        for th in pre[qc]:
                            th()
                    if len(pend) > 3:   # AV trails scores by 3 iterations
                        do_av(*pend.pop(0))
                    # fillers start at j==4: they may read avt columns
                    # that the j==3 deferred normalize produces
                    if nfl and j >= 4:
                        fj = j - 4
                        k0 = nfl * fj // (njt - 4)
                        k1 = nfl * (fj + 1) // (njt - 4)
                        for k in range(k0, k1):
                            fl[k]()
                for args in pend:
                    do_av(*args)
                norm_den(p, qc)

        project(0)
        project(1)
        attention(0,
                  pre=[[],
                       [lambda: norm_bc(0, 0)],
                       [lambda: norm_bc(0, 1)],
                       [lambda: norm_bc(0, 2)]],
                  fillers=[[], [], [], []])
        # pair-1 attention carries pair-0's last normalize, its own
        # normalizes, and the O projection (one qc behind)
        attention(1,
                  pre=[[lambda: norm_bc(0, 3)],
                       [lambda: norm_bc(1, 0)],
                       [lambda: norm_bc(1, 1)],
                       [lambda: norm_bc(1, 2)]],
                  fillers=[
                      [],
                      [lambda tt=t: o_unit(tt) for t in range(0, 4)],
                      [lambda tt=t: o_unit(tt) for t in range(4, 8)],
                      [lambda tt=t: o_unit(tt) for t in range(8, 12)],
                  ])
        norm_bc(1, 3)
        for tt in range(12, 16):
            o_unit(tt)
    nc.compile()
    return nc


def _get_built():
    global _BUILT
    if _BUILT is None:
        _BUILT = _build()
    return _BUILT


def _host_inputs(x, q_proj, k_proj, v_proj, o_proj):
    tri = np.triu(np.ones((128, 128), dtype=np.float32)).astype(BF)
    ident = np.eye(128, dtype=np.float32).astype(BF)
    xt = [np.ascontiguousarray(x[b].T).astype(BF) for b in range(B)]

    def wslice(w, gp):
        # [p, 8k x 128m]: w_sb[p, 128k+m] = w[128gp+m, 128k+p]
        a = w[128 * gp:128 * (gp + 1)].reshape(128, 8, 128)
        return np.ascontiguousarray(a.transpose(2, 1, 0).reshape(128, D))

    in_maps = []
    for c in range(NCORES):
        b, g4 = c // 4, c % 4
        gps = (2 * g4, 2 * g4 + 1)
        wq = np.stack([wslice(q_proj, gp) for gp in gps]).astype(BF)
        wk = np.stack([wslice(k_proj, gp) for gp in gps]).astype(BF)
        wv = np.stack([wslice(v_proj, gp) for gp in gps]).astype(BF)
        wo = np.stack(
            [np.ascontiguousarray(o_proj[:, 128 * gp:128 * (gp + 1)].T)
             for gp in gps]).astype(BF)
        in_maps.append(dict(xt=xt[b], wq=wq, wk=wk, wv=wv, wo=wo,
                            tri=tri, ident=ident,
                            ones1=np.ones((1, 64), dtype=np.float32)))
    return in_maps


def kernel(**inputs):
    x = np.asarray(inputs["x"], dtype=np.float32)
    q_proj = np.asarray(inputs["q_proj"], dtype=np.float32)
    k_proj = np.asarray(inputs["k_proj"], dtype=np.float32)
    v_proj = np.asarray(inputs["v_proj"], dtype=np.float32)
    o_proj = np.asarray(inputs["o_proj"], dtype=np.float32)

    in_maps = _host_inputs(x, q_proj, k_proj, v_proj, o_proj)
    nc = _get_built()
    global LAST_RESULTS
    LAST_RESULTS = run_bass_kernel_spmd(
        nc, in_maps, core_ids=list(range(NCORES)),
        trace=bool(os.environ.get("KERNEL_TRACE")))
    y = np.zeros((B, S, D), dtype=np.float32)
    for c in range(NCORES):
        y[c // 4] += np.asarray(LAST_RESULTS.results[c]["out"]).astype(
            np.float32)
    return y
